# revision 3
# baseline (speedup 1.0000x reference)
"""LinearAttention Trainium2 Bass kernel.

Full-input contract: kernel(**inputs) takes the unsharded inputs from
setup_inputs() and returns the full output. Internally shards data-parallel
over batch (b=8) across 8 NeuronCores; each core computes one batch item's
full linear attention:

  qkv = w_qkv @ x        (1x1 conv; on-chip layout B: [n, 768], n on partitions)
  q = softmax_d(q); k = softmax_n(k)
  ctx[h] = k_h @ v_h^T   (accumulated over n in PSUM)
  att[h] = ctx[h]^T @ q  (exp(q) normalized in layout B, DMA-xbar-transposed
                          to layout A)
  out = w_out @ att + b_out

Matmul operands are bf16 (fast FWL weight loads, 1 cyc/row streaming);
all accumulation stays fp32 in PSUM.
"""

import numpy as np
import ml_dtypes

import concourse.bass as bass
import concourse.tile as tile
from concourse import bacc, mybir
from concourse.bass_utils import run_bass_kernel_spmd

F32 = mybir.dt.float32
BF16 = mybir.dt.bfloat16
AF = mybir.ActivationFunctionType

C = 128          # input channels
N = 16384        # h*w
HEADS = 4
DH = 64          # dim_head
INNER = HEADS * DH          # 256
QKV = 3 * INNER             # 768
NB = 512                    # block width (columns of n)
SUB = NB // 128             # subtiles per block
NBLK = N // NB              # 32
NSUB = N // 128             # 128


def build_nc():
    nc = bacc.Bacc("TRN2", target_bir_lowering=False, debug=False, num_devices=8)

    x = nc.dram_tensor("x", [C, N], BF16, kind="ExternalInput")
    wqT = nc.dram_tensor("wqT", [C, QKV], BF16, kind="ExternalInput")
    woT = nc.dram_tensor("woT", [INNER, C], BF16, kind="ExternalInput")
    bo = nc.dram_tensor("bo", [C, 1], F32, kind="ExternalInput")
    out = nc.dram_tensor("out", [C, N], F32, kind="ExternalOutput")

    with tile.TileContext(nc) as tc:
        with (
            tc.tile_pool(name="consts", bufs=1) as consts,
            tc.tile_pool(name="eqa", bufs=1) as eqa,
            tc.tile_pool(name="xin", bufs=4) as xin,
            tc.tile_pool(name="work", bufs=4) as work,
            tc.tile_pool(name="small", bufs=4) as small,
        ):
            # ---- constants ----
            wq_s = consts.tile([C, QKV], BF16)
            nc.sync.dma_start(out=wq_s, in_=wqT[:, :])
            wo_s = consts.tile([C, 2, C], BF16)
            nc.sync.dma_start(out=wo_s[:, 0, :], in_=woT[0:128, :])
            nc.sync.dma_start(out=wo_s[:, 1, :], in_=woT[128:256, :])
            bo_s = consts.tile([C, 1], F32)
            nc.sync.dma_start(out=bo_s, in_=bo[:, :])

            # resident transposed normalized exp(q), layout A (d-pack on partitions)
            eqnA01 = eqa.tile([C, N], BF16)
            eqnA23 = eqa.tile([C, N], BF16)

            # ---- pass 1 ----
            with (
                tc.tile_pool(name="qkvp", bufs=3, space="PSUM") as qkvp,
                tc.tile_pool(name="ctxp", bufs=1, space="PSUM") as ctxp,
            ):
                ctx01 = ctxp.tile([C, INNER + 1], F32)
                ctx23 = ctxp.tile([C, INNER + 1], F32)

                x_blk = None
                for t in range(NSUB):
                    blk, s = divmod(t, SUB)
                    if s == 0:
                        x_blk = xin.tile([C, NB], BF16, tag="x_blk")
                        nc.sync.dma_start(
                            out=x_blk, in_=x[:, blk * NB : (blk + 1) * NB]
                        )
                    xs = x_blk[:, s * 128 : (s + 1) * 128]

                    qkv = qkvp.tile([C, QKV], F32, tag="qkv")
                    nc.tensor.matmul(
                        qkv[:, 0:512], lhsT=xs, rhs=wq_s[:, 0:512],
                        start=True, stop=True, skip_group_check=True,
                    )
                    nc.tensor.matmul(
                        qkv[:, 512:768], lhsT=xs, rhs=wq_s[:, 512:768],
                        start=True, stop=True, skip_group_check=True,
                    )

                    # exp over q and k halves in one ACT op -> bf16
                    # heads 0..3 = q, 4..7 = k
                    eqk = work.tile([C, 8, DH], BF16, tag="eqk")
                    nc.scalar.activation(eqk[:, :, :], qkv[:, 0:512], AF.Exp)

                    # q softmax over d (free dim here): sum, recip, scale
                    sq = small.tile([C, HEADS, 1], F32, tag="sq")
                    nc.vector.reduce_sum(
                        sq, eqk[:, 0:4, :], axis=mybir.AxisListType.X
                    )
                    rq = small.tile([C, HEADS, 1], F32, tag="rq")
                    nc.vector.reciprocal(rq, sq)
                    eqn = work.tile([C, HEADS, DH], BF16, tag="eqn")
                    nc.vector.tensor_mul(
                        eqn, eqk[:, 0:4, :], rq.broadcast_to([C, HEADS, DH])
                    )

                    # v (+ ones column for sum_n exp(k))
                    vt = work.tile([C, INNER + 1], BF16, tag="vt")
                    if t % 2 == 0:
                        nc.vector.tensor_copy(vt[:, 0:256], qkv[:, 512:768])
                    else:
                        nc.scalar.copy(vt[:, 0:256], qkv[:, 512:768])
                    nc.gpsimd.memset(vt[:, 256:257], 1.0)

                    # context accumulation (head pairs packed on lhsT cols)
                    nc.tensor.matmul(
                        ctx01, lhsT=eqk[:, 4:6, :], rhs=vt,
                        start=(t == 0), stop=(t == NSUB - 1), skip_group_check=True,
                    )
                    nc.tensor.matmul(
                        ctx23, lhsT=eqk[:, 6:8, :], rhs=vt,
                        start=(t == 0), stop=(t == NSUB - 1), skip_group_check=True,
                    )

                    # transpose eqn to layout A via DMA xbar (bf16)
                    nc.sync.dma_start_transpose(
                        eqnA01[:, t * 128 : (t + 1) * 128], eqn[:, 0:2, :]
                    )
                    nc.sync.dma_start_transpose(
                        eqnA23[:, t * 128 : (t + 1) * 128], eqn[:, 2:4, :]
                    )

                # ---- finalize context: divide by s_k, build block-diag lhsT ----
                lhsT01 = consts.tile([C, C], BF16)
                lhsT23 = consts.tile([C, C], BF16)
                r01 = small.tile([C, 1], F32, tag="r01")
                r23 = small.tile([C, 1], F32, tag="r23")
                nc.vector.reciprocal(r01, ctx01[:, 256:257])
                nc.vector.reciprocal(r23, ctx23[:, 256:257])
                nc.vector.tensor_scalar_mul(
                    lhsT01[0:64, 0:64], ctx01[0:64, 0:64], r01[0:64, 0:1]
                )
                nc.vector.tensor_scalar_mul(
                    lhsT01[64:128, 64:128], ctx01[64:128, 64:128], r01[64:128, 0:1]
                )
                nc.vector.tensor_scalar_mul(lhsT01[0:64, 64:128], ctx01[0:64, 64:128], 0.0)
                nc.vector.tensor_scalar_mul(lhsT01[64:128, 0:64], ctx01[64:128, 0:64], 0.0)
                nc.vector.tensor_scalar_mul(
                    lhsT23[0:64, 0:64], ctx23[0:64, 128:192], r23[0:64, 0:1]
                )
                nc.vector.tensor_scalar_mul(
                    lhsT23[64:128, 64:128], ctx23[64:128, 192:256], r23[64:128, 0:1]
                )
                nc.vector.tensor_scalar_mul(lhsT23[0:64, 64:128], ctx23[0:64, 0:64], 0.0)
                nc.vector.tensor_scalar_mul(lhsT23[64:128, 0:64], ctx23[64:128, 0:64], 0.0)

            # ---- pass 2 ----
            with (
                tc.tile_pool(name="outp", bufs=2, space="PSUM") as outp,
                tc.tile_pool(name="finp", bufs=2, space="PSUM") as finp,
            ):
                for blk in range(NBLK):
                    nsl = slice(blk * NB, (blk + 1) * NB)
                    o01 = outp.tile([C, NB], F32, tag="o01")
                    o23 = outp.tile([C, NB], F32, tag="o23")
                    nc.tensor.matmul(
                        o01, lhsT=lhsT01, rhs=eqnA01[:, nsl],
                        start=True, stop=True, skip_group_check=True,
                    )
                    nc.tensor.matmul(
                        o23, lhsT=lhsT23, rhs=eqnA23[:, nsl],
                        start=True, stop=True, skip_group_check=True,
                    )
                    att01 = work.tile([C, NB], BF16, tag="att01")
                    att23 = work.tile([C, NB], BF16, tag="att23")
                    nc.vector.tensor_copy(att01, o01)
                    nc.scalar.copy(att23, o23)
                    fin = finp.tile([C, NB], F32, tag="fin")
                    nc.tensor.matmul(
                        fin, lhsT=wo_s[:, 0, :], rhs=att01,
                        start=True, stop=False, skip_group_check=True,
                    )
                    nc.tensor.matmul(
                        fin, lhsT=wo_s[:, 1, :], rhs=att23,
                        start=False, stop=True, skip_group_check=True,
                    )
                    osb = work.tile([C, NB], F32, tag="osb")
                    if blk % 2 == 0:
                        nc.scalar.activation(
                            osb, fin, AF.Identity, bias=bo_s[:, 0:1], scale=1.0
                        )
                    else:
                        nc.vector.tensor_scalar_add(osb, fin, bo_s[:, 0:1])
                    nc.sync.dma_start(out=out[:, nsl], in_=osb)

    nc.compile()
    return nc


_NC_CACHE = None


def kernel(x, w_qkv, w_out, b_out):
    global _NC_CACHE
    if _NC_CACHE is None:
        _NC_CACHE = build_nc()
    nc = _NC_CACHE

    b = x.shape[0]
    bf = ml_dtypes.bfloat16
    wqT = np.ascontiguousarray(np.asarray(w_qkv, dtype=np.float32).T.astype(bf))
    woT = np.ascontiguousarray(np.asarray(w_out, dtype=np.float32).T.astype(bf))
    bo = np.ascontiguousarray(np.asarray(b_out, dtype=np.float32).reshape(C, 1))
    xb = np.asarray(x, dtype=np.float32).reshape(b, C, N).astype(bf)
    in_maps = [
        {"x": np.ascontiguousarray(xb[i]), "wqT": wqT, "woT": woT, "bo": bo}
        for i in range(b)
    ]
    res = run_bass_kernel_spmd(nc, in_maps, core_ids=list(range(b)))
    out = np.stack(
        [res.results[i]["out"].reshape(C, 128, 128) for i in range(b)]
    ).astype(np.float32)
    return out


# revision 5
# speedup vs baseline: 1.9612x; 1.9612x over previous
"""LinearAttention Trainium2 Bass kernel.

kernel(**inputs) takes the full unsharded inputs from setup_inputs() and
returns the full output. Shards data-parallel over batch (b=8) across 8
NeuronCores; each core computes one batch item:

  qkv = w_qkv @ x            (layout B on chip: [n, 768], n on partitions)
  q = softmax_d(q); k = softmax_n(k)
  ctx[h] = ek_h^T @ v_h      (accumulated over n in PSUM; an appended ones
                              column yields sum_n ek for the k softmax)
  M^T = blockdiag(ctx/s_k)^T @ w_out^T   (folded once between passes)
  out = M @ eqnA + b_out     (eqnA = PE-transposed normalized exp(q))

Matmul operands are bf16; accumulation stays fp32 in PSUM.
"""

import numpy as np
import ml_dtypes

import concourse.bass as bass
import concourse.tile as tile
from concourse import bacc, mybir
from concourse.bass_utils import run_bass_kernel_spmd
from concourse.masks import make_identity

F32 = mybir.dt.float32
BF16 = mybir.dt.bfloat16
AF = mybir.ActivationFunctionType

C = 128
N = 16384
HEADS = 4
DH = 64
INNER = HEADS * DH          # 256
QKV = 3 * INNER             # 768
NB = 512
SUB = NB // 128
NBLK = N // NB              # 32
NSUB = N // 128             # 128


def build_nc():
    nc = bacc.Bacc("TRN2", target_bir_lowering=False, debug=False, num_devices=8)

    x = nc.dram_tensor("x", [C, N], BF16, kind="ExternalInput")
    wqT = nc.dram_tensor("wqT", [C, QKV], BF16, kind="ExternalInput")
    woT = nc.dram_tensor("woT", [INNER, C], BF16, kind="ExternalInput")
    bo = nc.dram_tensor("bo", [C, 1], F32, kind="ExternalInput")
    out = nc.dram_tensor("out", [C, N], F32, kind="ExternalOutput")

    with tile.TileContext(nc) as tc:
        with (
            tc.tile_pool(name="consts", bufs=1) as consts,
            tc.tile_pool(name="eqa", bufs=1) as eqa,
            tc.tile_pool(name="xin", bufs=4) as xin,
            tc.tile_pool(name="work", bufs=4) as work,
            tc.tile_pool(name="small", bufs=4) as small,
        ):
            wq_s = consts.tile([C, QKV], BF16)
            nc.sync.dma_start(out=wq_s, in_=wqT[:, :])
            wo_s = consts.tile([C, 2, C], BF16)
            nc.sync.dma_start(out=wo_s[:, 0, :], in_=woT[0:128, :])
            nc.sync.dma_start(out=wo_s[:, 1, :], in_=woT[128:256, :])
            bo_s = consts.tile([C, 1], F32)
            nc.sync.dma_start(out=bo_s, in_=bo[:, :])
            ident = consts.tile([C, C], BF16)
            make_identity(nc, ident)

            # layout-A normalized exp(q): [:, 0, :] = heads 0/1, [:, 1, :] = 2/3
            eqnA = eqa.tile([C, 2, N], BF16)
            MT01 = consts.tile([C, C], BF16)
            MT23 = consts.tile([C, C], BF16)

            with (
                tc.tile_pool(name="qkvp", bufs=2, space="PSUM") as qkvp,
                tc.tile_pool(name="trp", bufs=2, space="PSUM") as trp,
                tc.tile_pool(name="ctxp", bufs=1, space="PSUM") as ctxp,
            ):
                ctx01 = ctxp.tile([C, INNER + 1], F32)
                ctx23 = ctxp.tile([C, INNER + 1], F32)

                x_blk = None
                for t in range(NSUB):
                    blk, s = divmod(t, SUB)
                    if s == 0:
                        x_blk = xin.tile([C, NB], BF16, tag="x_blk")
                        nc.sync.dma_start(
                            out=x_blk, in_=x[:, blk * NB : (blk + 1) * NB]
                        )
                    xs = x_blk[:, s * 128 : (s + 1) * 128]

                    qkv = qkvp.tile([C, QKV], F32, tag="qkv")
                    nc.tensor.matmul(
                        qkv[:, 0:512], lhsT=xs, rhs=wq_s[:, 0:512],
                        start=True, stop=True, skip_group_check=True,
                    )
                    nc.tensor.matmul(
                        qkv[:, 512:768], lhsT=xs, rhs=wq_s[:, 512:768],
                        start=True, stop=True, skip_group_check=True,
                    )

                    # one exp over q|k halves; heads 0..3 = q, 4..7 = k
                    eqk = work.tile([C, 8, DH], BF16, tag="eqk")
                    nc.scalar.activation(eqk[:, :, :], qkv[:, 0:512], AF.Exp)

                    sq = small.tile([C, HEADS, 1], F32, tag="sq")
                    nc.vector.reduce_sum(
                        sq, eqk[:, 0:4, :], axis=mybir.AxisListType.X
                    )
                    rq = small.tile([C, HEADS, 1], F32, tag="rq")
                    nc.vector.reciprocal(rq, sq)
                    eqn = work.tile([C, HEADS, DH], BF16, tag="eqn")
                    nc.gpsimd.tensor_mul(
                        eqn, eqk[:, 0:4, :], rq.broadcast_to([C, HEADS, DH])
                    )

                    vt = work.tile([C, INNER + 1], BF16, tag="vt")
                    if t % 2 == 0:
                        nc.vector.tensor_copy(vt[:, 0:256], qkv[:, 512:768])
                    else:
                        nc.scalar.copy(vt[:, 0:256], qkv[:, 512:768])
                    nc.gpsimd.memset(vt[:, 256:257], 1.0)

                    nc.tensor.matmul(
                        ctx01, lhsT=eqk[:, 4:6, :], rhs=vt,
                        start=(t == 0), stop=(t == NSUB - 1), skip_group_check=True,
                    )
                    nc.tensor.matmul(
                        ctx23, lhsT=eqk[:, 6:8, :], rhs=vt,
                        start=(t == 0), stop=(t == NSUB - 1), skip_group_check=True,
                    )

                    tr = trp.tile([C, 2, C], BF16, tag="tr")
                    nc.tensor.transpose(tr[:, 0, :], eqn[:, 0:2, :], ident)
                    nc.tensor.transpose(tr[:, 1, :], eqn[:, 2:4, :], ident)
                    if t % 2 == 0:
                        nc.scalar.copy(eqnA[:, :, t * 128 : (t + 1) * 128], tr)
                    else:
                        nc.vector.tensor_copy(
                            eqnA[:, :, t * 128 : (t + 1) * 128], tr
                        )

                # ---- fold: MT = (blockdiag(ctx/s_k))^T @ w_out^T ----
                r01 = small.tile([C, 1], F32, tag="r01")
                r23 = small.tile([C, 1], F32, tag="r23")
                nc.vector.reciprocal(r01, ctx01[:, 256:257])
                nc.vector.reciprocal(r23, ctx23[:, 256:257])
                bd01 = consts.tile([C, C], BF16)
                bd23 = consts.tile([C, C], BF16)
                nc.vector.tensor_scalar_mul(
                    bd01[0:64, 0:64], ctx01[0:64, 0:64], r01[0:64, 0:1]
                )
                nc.vector.tensor_scalar_mul(
                    bd01[64:128, 64:128], ctx01[64:128, 64:128], r01[64:128, 0:1]
                )
                nc.vector.tensor_scalar_mul(bd01[0:64, 64:128], ctx01[0:64, 64:128], 0.0)
                nc.vector.tensor_scalar_mul(bd01[64:128, 0:64], ctx01[64:128, 0:64], 0.0)
                nc.vector.tensor_scalar_mul(
                    bd23[0:64, 0:64], ctx23[0:64, 128:192], r23[0:64, 0:1]
                )
                nc.vector.tensor_scalar_mul(
                    bd23[64:128, 64:128], ctx23[64:128, 192:256], r23[64:128, 0:1]
                )
                nc.vector.tensor_scalar_mul(bd23[0:64, 64:128], ctx23[0:64, 0:64], 0.0)
                nc.vector.tensor_scalar_mul(bd23[64:128, 0:64], ctx23[64:128, 0:64], 0.0)

                for pair, bd, mt in ((0, bd01, MT01), (1, bd23, MT23)):
                    tb = trp.tile([C, 2, C], BF16, tag="tr")
                    nc.tensor.transpose(tb[:, 0, :], bd, ident)
                    bdt = consts.tile([C, C], BF16, tag=f"bdt{pair}")
                    nc.vector.tensor_copy(bdt, tb[:, 0, :])
                    mtp = qkvp.tile([C, QKV], F32, tag="qkv")
                    nc.tensor.matmul(
                        mtp[:, 0:128], lhsT=bdt, rhs=wo_s[:, pair, :],
                        start=True, stop=True, skip_group_check=True,
                    )
                    nc.vector.tensor_copy(mt, mtp[:, 0:128])

            # ---- pass 2: out = MT^T @ eqnA + b ----
            with tc.tile_pool(name="finp", bufs=2, space="PSUM") as finp:
                for blk in range(NBLK):
                    nsl = slice(blk * NB, (blk + 1) * NB)
                    fin = finp.tile([C, NB], F32, tag="fin")
                    nc.tensor.matmul(
                        fin, lhsT=MT01, rhs=eqnA[:, 0, nsl],
                        start=True, stop=False, skip_group_check=True,
                    )
                    nc.tensor.matmul(
                        fin, lhsT=MT23, rhs=eqnA[:, 1, nsl],
                        start=False, stop=True, skip_group_check=True,
                    )
                    osb = work.tile([C, NB], F32, tag="osb")
                    if blk % 2 == 0:
                        nc.scalar.activation(
                            osb, fin, AF.Identity, bias=bo_s[:, 0:1], scale=1.0
                        )
                    else:
                        nc.vector.tensor_scalar_add(osb, fin, bo_s[:, 0:1])
                    nc.sync.dma_start(out=out[:, nsl], in_=osb)

    nc.compile()
    return nc


_NC_CACHE = None


def kernel(x, w_qkv, w_out, b_out):
    global _NC_CACHE
    if _NC_CACHE is None:
        _NC_CACHE = build_nc()
    nc = _NC_CACHE

    b = x.shape[0]
    bf = ml_dtypes.bfloat16
    wqT = np.ascontiguousarray(np.asarray(w_qkv, dtype=np.float32).T.astype(bf))
    woT = np.ascontiguousarray(np.asarray(w_out, dtype=np.float32).T.astype(bf))
    bo = np.ascontiguousarray(np.asarray(b_out, dtype=np.float32).reshape(C, 1))
    xb = np.asarray(x, dtype=np.float32).reshape(b, C, N).astype(bf)
    in_maps = [
        {"x": np.ascontiguousarray(xb[i]), "wqT": wqT, "woT": woT, "bo": bo}
        for i in range(b)
    ]
    res = run_bass_kernel_spmd(nc, in_maps, core_ids=list(range(b)))
    return np.stack(
        [res.results[i]["out"].reshape(C, 128, 128) for i in range(b)]
    ).astype(np.float32)
